# revision 25
# baseline (speedup 1.0000x reference)
"""BSBR attention kernel for 8 Trainium2 NeuronCores.

Sharding: data-parallel over batch (B=2) x tensor-parallel over heads
(16 heads -> 4 heads per core). Core c handles batch c//4, head group c%4.
Each core computes its 4 heads' attention output and the partial output
projection (attn_heads @ Wo[:, head_dims].T); the host sums the 4 partials
per batch (bf16 partials, f32 accumulate) and adds bo.

Schedule (v2):
- Phase 1: x arrives as 8 x 1MB descriptors (s-slice major) on the sync
  queue; weights on the gpsimd queue. Per slice n: Q^T projection, fused
  V|K-natural projection (rank-1 bias trick), K^T recovered from K-natural
  by PE transposes (saves a whole D x DHC projection), and the per-chunk
  F = K^T V matmuls. F rows stream to DRAM in two pipelined half-bounces.
- Interlude: r/h meta projections, chunk softmax, retrieval mix (PSUM
  reuses the out-projection bank), retrieved-rows bounce to retrT layout.
- Phase 2: per-chunk 3-stage pipeline. A: S^T = K Q^T, exp on ACT, binary
  mask on DVE. B: U matmul (ones-column rank-1 rowsum), LT matmul,
  combine via scalar_tensor_tensor. C: PE transpose of attn to attn^T,
  output projection, bf16 DMA out. B/C trail A by RETR_LEAD chunks so the
  retrieved-state bounce never stalls the PE; PSUM packed into 8 banks.
"""

import numpy as np

try:
    import concourse.bass as bass
except ImportError:
    import sys

    sys.path.insert(0, "/opt/trn_rl_repo")
    import concourse.bass as bass

import ml_dtypes
from contextlib import ExitStack

import concourse.tile as tile
from concourse import mybir
from concourse.bass_utils import run_bass_kernel_spmd

BF16 = ml_dtypes.bfloat16
B, S, D, H, CS = 2, 4096, 1024, 16, 128
HD = D // H          # 64
C = S // CS          # 32
NCORES = 8
DHC = 4 * HD         # 256 head dims per core
KB = D // 128        # 8 contraction blocks
NEG = -1e9
RETR_LEAD = 22       # chunks stage A runs ahead of B/C

bf = mybir.dt.bfloat16
f32 = mybir.dt.float32
Exp = mybir.ActivationFunctionType.Exp
MULT = mybir.AluOpType.mult
ADD = mybir.AluOpType.add

# head -> (psum bank group, slot); in-bank pairs share PE rows => serial.
POS = {0: (0, 0), 2: (0, 1), 1: (1, 0), 3: (1, 1)}
HORD = (0, 2, 1, 3)


def _split_heavy_waits(nc, keep=1):
    """The walrus build in this container rejects >keep sync waits on several
    instruction encodings. Hoist excess waits onto preceding NoOps on the
    same engine — the sequencer executes them in order."""
    for fn in nc.m.functions:
        for bb in fn.blocks:
            insts = bb.instructions
            i = 0
            while i < len(insts):
                inst = insts[i]
                si = inst.sync_info
                if si is not None and si.on_wait and len(si.on_wait) > keep:
                    waits = list(si.on_wait)
                    head, tail = waits[:-keep], waits[-keep:]
                    for j, w in enumerate(head):
                        nop = mybir.InstNoOp(
                            name=nc.get_next_instruction_name(), ins=[], outs=[]
                        )
                        nop.engine = inst.engine
                        nop.sync_info = mybir.SyncInfo(on_wait=[w], on_update=[])
                        nc.register_instruction(nop, overwrite=True)
                        insts.insert(i + j, nop)
                    inst.sync_info = mybir.SyncInfo(
                        on_wait=tail, on_update=list(si.on_update)
                    )
                    i += len(head)
                i += 1


def _build_program():
    nc = bass.Bass("TRN2", debug=False, num_devices=NCORES)

    ap = {}
    def din(name, shape, dtype):
        ap[name] = nc.dram_tensor(name, shape, dtype, kind="ExternalInput").ap()

    din("xT", [D, S], bf)
    for w in ("wqT", "wkT", "wvT", "wrT", "whT"):
        din(w, [D, DHC], bf)
    din("woT", [DHC, D], bf)
    din("bias", [128, 8], f32)
    din("bvkrow", [1, 512], bf)
    din("ident", [128, 128], bf)
    din("maskb4", [128, 512], bf)   # binary local mask, [key, query] tiled x4
    din("cmask4", [128, C], f32)    # additive chunk mask [j, c'] tiled x4
    out_ap = nc.dram_tensor("out", [S, D], bf, kind="ExternalOutput").ap()

    with tile.TileContext(nc) as tc, ExitStack() as ctx:
        const = ctx.enter_context(tc.tile_pool(name="const", bufs=1))
        wpool = ctx.enter_context(tc.tile_pool(name="wpool", bufs=1))
        big = ctx.enter_context(tc.tile_pool(name="big", bufs=1))
        dram = ctx.enter_context(tc.tile_pool(name="dramp", bufs=1, space="DRAM"))

        # ---- constants (gpsimd queue, after the critical wvk) ----
        bias_sb = const.tile([128, 8], f32)
        bvkrow_sb = const.tile([1, 512], bf)
        onesrow_sb = const.tile([1, 128], bf)
        nc.vector.memset(onesrow_sb[:], 1.0)
        onescol_sb = const.tile([128, 1], bf)
        nc.vector.memset(onescol_sb[:], 1.0)
        warm_sb = const.tile([128, 512], bf)
        nc.vector.memset(warm_sb[:], 0.0)

        # ---- weights: critical path (wq, wvk, ident) split across the
        # scalar and gpsimd queues; late-use weights ride the sync queue
        # behind the x slices ----
        # wq halves by m so the first Q chain only waits on 256KB; wvk rides
        # the scalar/gpsimd queues (k-halves) behind the first x quarters
        wq_sb = wpool.tile([128, KB, DHC], bf)
        wqsrc = ap["wqT"].rearrange("(k p) d -> p k d", p=128)
        nc.sync.dma_start(wq_sb[:, :, 0:128], wqsrc[:, :, 0:128])
        nc.sync.dma_start(wq_sb[:, :, 128:256], wqsrc[:, :, 128:256])
        wvk_sb = wpool.tile([128, KB, 512], bf)
        ident_sb = const.tile([128, 128], bf)
        maskb_sb = const.tile([128, 512], bf)
        cmask_sb = const.tile([128, C], f32)
        wr_sb = wpool.tile([128, KB, DHC], bf)
        wh_sb = wpool.tile([128, KB, DHC], bf)
        wo_sb = wpool.tile([128, 2, D], bf)

        # ---- persistent activations ----
        qt_sb = [big.tile([128, S], bf, name=f"qt{m}") for m in range(2)]
        kt_sb = [big.tile([128, S], bf, name=f"kt{m}") for m in range(2)]
        v_sb = [big.tile([128, 4 * 65], bf, name=f"v{i}") for i in range(C)]
        # ones columns of v (rank-1 rowsum trick) never change: write once now,
        # while DVE is idle waiting on the first x slice
        for i in range(C):
            nc.vector.memset(
                v_sb[i].rearrange("p (h e) -> p h e", e=65)[:, :, 64:65], 1.0
            )
        knat_sb = [big.tile([128, DHC], bf, name=f"kn{i}") for i in range(C)]
        rt_sb = big.tile([128, 2, C], bf)
        ht_sb = big.tile([128, 2, C], bf)
        # chunk-probs stored block-diagonally: head h occupies the (32h, 32h)
        # 32x32 block, zeros elsewhere kill cross-head terms so the retrieval
        # mix and chunk rowsums run as full-128-contraction matmuls
        expbd_sb = big.tile([128, 128], bf)
        nc.vector.memset(expbd_sb[:], 0.0)
        crecip_sb = big.tile([128, 1], f32)
        fnat_sb = [big.tile([128, C * 64], bf, name=f"fnat{p}") for p in range(2)]
        frows_sb = big.tile([128, 64 * 64], bf)
        retrrows_sb = big.tile([128, 64 * 64], bf)
        crepr_sb = big.tile([128, KB, C], bf)
        fb = dram.tile([2, 2, 64, C, 64], bf)    # (pair, h2, d, c, e)
        rbt = dram.tile([4, C, 64, 64], bf)      # (head, c', d, e)

        # ---- phase 1: load x, projections, K^T transposes, F ----
        with tc.tile_pool(name="xtp", bufs=1) as xtpool:
            # one tile per s-slice: a matmul on slice n only waits for that
            # slice's DMA, not the whole 8MB load
            xts = [xtpool.tile([128, KB, 512], bf, name=f"xt{n}") for n in range(8)]
            xsrc = ap["xT"].rearrange("(k p) s -> p k s", p=128)
            # tiny constants lead gpsimd (bias needed at the first Q evac)
            nc.gpsimd.dma_start(bias_sb[:], ap["bias"][:])
            nc.gpsimd.dma_start(bvkrow_sb[:], ap["bvkrow"][:])
            nc.gpsimd.dma_start(ident_sb[:], ap["ident"][:])
            # first slice as interleaved quarters on the scalar+gpsimd queues:
            # cols 0:256 land after one quarter per queue, so the first (split)
            # Q chain starts one queue-quarter earlier than a half-split would
            for (c0, c1), eng in (
                ((0, 128), nc.scalar),
                ((128, 256), nc.gpsimd),
                ((256, 384), nc.scalar),
                ((384, 512), nc.gpsimd),
            ):
                eng.dma_start(xts[0][:, :, c0:c1], xsrc[:, :, c0:c1])
            # wvk k-halves behind the x quarters on the two side queues
            nc.scalar.dma_start(
                wvk_sb[:, 0:4, 0:DHC],
                ap["wvT"].rearrange("(k p) d -> p k d", p=128)[:, 0:4, :],
            )
            nc.scalar.dma_start(
                wvk_sb[:, 0:4, DHC:512],
                ap["wkT"].rearrange("(k p) d -> p k d", p=128)[:, 0:4, :],
            )
            nc.gpsimd.dma_start(
                wvk_sb[:, 4:8, 0:DHC],
                ap["wvT"].rearrange("(k p) d -> p k d", p=128)[:, 4:8, :],
            )
            nc.gpsimd.dma_start(
                wvk_sb[:, 4:8, DHC:512],
                ap["wkT"].rearrange("(k p) d -> p k d", p=128)[:, 4:8, :],
            )
            for n in range(1, 8):
                nc.sync.dma_start(
                    xts[n][:], xsrc[:, :, n * 512 : (n + 1) * 512]
                )
            nc.sync.dma_start(wr_sb[:], ap["wrT"].rearrange("(k p) d -> p k d", p=128))
            nc.sync.dma_start(wh_sb[:], ap["whT"].rearrange("(k p) d -> p k d", p=128))
            nc.sync.dma_start(wo_sb[:], ap["woT"].rearrange("(k p) j -> p k j", p=128))
            nc.sync.dma_start(maskb_sb[:], ap["maskb4"][:])
            nc.sync.dma_start(cmask_sb[:], ap["cmask4"][:])

            with (
                tc.tile_pool(name="pjp", bufs=2, space="PSUM") as pjp,
                tc.tile_pool(name="pvp", bufs=2, space="PSUM") as pvp,
                tc.tile_pool(name="ktp", bufs=2, space="PSUM") as ktpp,
                tc.tile_pool(name="fps", bufs=2, space="PSUM") as fpsp,
            ):
                # HAM warm-up: the PE clock gate defaults to 1.2 GHz and only
                # opens to 2.4 GHz after ~3.4us of sustained activity. Burn
                # dummy matmuls on a zeroed scratch during the initial DMA
                # wait so the real projections start at full clock.
                wups = pjp.tile([128, 512], f32, tag="pj")
                for w in range(35):
                    nc.tensor.matmul(
                        wups[:], warm_sb[:, 0:128], warm_sb[:],
                        start=True, stop=True, skip_group_check=True,
                    )
                def emit_q(n, ms=(0, 1), crs=((0, 512),)):
                    for m in ms:
                        ps = pjp.tile([128, 512], f32, tag="pj")
                        for c0, c1 in crs:
                            for k in range(KB):
                                nc.tensor.matmul(
                                    ps[:, c0:c1],
                                    wq_sb[:, k, m * 128 : (m + 1) * 128],
                                    xts[n][:, k, c0:c1],
                                    start=(k == 0),
                                    stop=(k == KB - 1),
                                )
                        nc.scalar.add(
                            qt_sb[m][:, n * 512 : (n + 1) * 512],
                            ps[:],
                            bias_sb[:, m : m + 1],
                        )

                def emit_vkf(n):
                    # V + K natural [s, dh] in one N=512 matmul chain; biases
                    # via a rank-1 matmul with [bv | bk]
                    for i in range(4 * n, 4 * n + 4):
                        ps = pvp.tile([128, 512], f32, tag="pv")
                        nc.tensor.matmul(
                            ps[:], onesrow_sb[:], bvkrow_sb[:],
                            start=True, stop=False, skip_group_check=True,
                        )
                        for k in range(KB):
                            nc.tensor.matmul(
                                ps[:],
                                xts[i // 4][:, k, (i % 4) * 128 : (i % 4 + 1) * 128],
                                wvk_sb[:, k, :],
                                start=False,
                                stop=(k == KB - 1),
                                skip_group_check=True,
                            )
                        vr = v_sb[i].rearrange("p (h e) -> p h e", e=65)
                        nc.vector.tensor_copy(
                            vr[:, :, 0:64],
                            ps[:, 0:DHC].rearrange("p (h e) -> p h e", e=64),
                        )
                        nc.vector.tensor_copy(knat_sb[i][:], ps[:, DHC:512])
                    # K^T via PE transposes + per-chunk F = k^T v. F matmuls
                    # pair heads: full-width stationary (FWL) over a head pair
                    # computes both diag blocks (plus discarded off-diag) in
                    # one N=128 stream; evacuation picks the diag blocks.
                    for i in range(4 * n, 4 * n + 4):
                        ktp = ktpp.tile([128, 256], bf, tag="ktp")
                        for m in range(2):
                            nc.tensor.transpose(
                                ktp[:, m * 128 : (m + 1) * 128],
                                knat_sb[i][:, m * 128 : (m + 1) * 128],
                                ident_sb[:],
                            )
                        fps = fpsp.tile([128, 256], f32, tag="fps")
                        vr = v_sb[i].rearrange("p (h e) -> p h e", e=65)
                        for p in range(2):
                            nc.tensor.matmul(
                                fps[:, p * 128 : (p + 1) * 128],
                                knat_sb[i][:, p * 128 : (p + 1) * 128],
                                vr[:, 2 * p : 2 * p + 2, 0:64],
                                start=True, stop=True, skip_group_check=True,
                            )
                        for m in range(2):
                            nc.scalar.copy(
                                kt_sb[m][:, i * 128 : (i + 1) * 128],
                                ktp[:, m * 128 : (m + 1) * 128],
                            )
                        for p in range(2):
                            for h2 in range(2):
                                nc.vector.tensor_copy(
                                    fnat_sb[p][64 * h2 : 64 * h2 + 64,
                                               i * 64 : (i + 1) * 64],
                                    fps[64 * h2 : 64 * h2 + 64,
                                        p * 128 + 64 * h2 : p * 128 + 64 * h2 + 64],
                                )

                def emit_fbw(c0, c1):
                    for p in range(2):
                        for h2 in range(2):
                            nc.scalar.dma_start(
                                fb[p, h2][:, c0:c1, :],
                                fnat_sb[p][64 * h2 : 64 * h2 + 64,
                                           c0 * 64 : c1 * 64].rearrange(
                                    "d (c e) -> d c e", e=64
                                ),
                            )

                def emit_frr(c0, c1):
                    for h in range(4):
                        p, h2 = divmod(h, 2)
                        nc.gpsimd.dma_start(
                            frows_sb[32 * h + c0 : 32 * h + c1, :].rearrange(
                                "c (d e) -> c d e", e=64
                            ),
                            fb[p, h2][:, c0:c1, :].rearrange("d c e -> c d e"),
                        )

                def emit_crepr(n):
                    nc.gpsimd.tensor_copy(
                        crepr_sb[:, :, 4 * n : 4 * n + 4],
                        xts[n].rearrange("p k (c cs) -> p k c cs", cs=CS)[
                            :, :, :, CS - 1
                        ],
                    )

                for n in range(7):
                    emit_q(n, crs=((0, 256), (256, 512)) if n == 0 else ((0, 512),))
                    emit_vkf(n)
                    emit_crepr(n)
                    if n in (1, 3, 5):
                        emit_fbw((n - 1) * 4, (n + 1) * 4)
                    if n == 6:
                        emit_fbw(24, 28)
                    if n in (2, 4, 6):
                        emit_frr((n - 2) * 4, n * 4)

                # final slice: VK/F first so the F bounce completes early,
                # then r/h + chunk softmax + retrieval mix overlap Q(7)
                emit_frr(24, 28)
                emit_vkf(7)
                emit_crepr(7)
                emit_fbw(28, 32)
                emit_frr(28, 32)

                # r/h meta projections: [dh, c] layout
                for w_sb, dst, bcol in ((wr_sb, rt_sb, 4), (wh_sb, ht_sb, 6)):
                    for m in range(2):
                        ps = pjp.tile([128, 512], f32, tag="pj")
                        for k in range(KB):
                            nc.tensor.matmul(
                                ps[:, 0:C],
                                w_sb[:, k, m * 128 : (m + 1) * 128],
                                crepr_sb[:, k, :],
                                start=(k == 0),
                                stop=(k == KB - 1),
                            )
                        nc.scalar.add(
                            dst[:, m, :], ps[:, 0:C],
                            bias_sb[:, bcol + m : bcol + m + 1],
                        )

                # Q(7) m=0 chain covers the r/h bias-add latency
                emit_q(7, ms=(0,))

                # chunk softmax (PSUM scratch reuses the pv tag); exp lands in
                # the block-diag expbd tile (zeros pre-cleared)
                cs = pvp.tile([128, 512], f32, tag="pv")
                for h in range(4):
                    hb = 64 * (h % 2)
                    nc.tensor.matmul(
                        cs[32 * h : 32 * h + 32, 0:C],
                        ht_sb[hb : hb + 64, h // 2, :],
                        rt_sb[hb : hb + 64, h // 2, :],
                        start=True, stop=True, skip_group_check=True,
                        tile_position=(hb, 32 * h),
                    )
                nc.vector.tensor_add(cs[:, 0:C], cs[:, 0:C], cmask_sb[:])
                for h in range(4):
                    nc.scalar.activation(
                        expbd_sb[32 * h : 32 * h + 32, 32 * h : 32 * h + 32],
                        cs[32 * h : 32 * h + 32, 0:C], Exp, scale=0.125,
                    )
                # Q(7) m=1 chain covers the cmask-add + exp roundtrip
                emit_q(7, ms=(1,))
                nc.tensor.matmul(
                    cs[:, C : C + 1], expbd_sb[:], onescol_sb[:],
                    start=True, stop=True, skip_group_check=True,
                )
                nc.vector.reciprocal(crecip_sb[:], cs[:, C : C + 1])

                # retrieval mix: full-contraction matmuls off the block-diag
                # probs (PSUM reuses the pj tag)
                for nb in range(8):
                    mps = pjp.tile([128, 512], f32, tag="pj")
                    nc.tensor.matmul(
                        mps[:], expbd_sb[:],
                        frows_sb[:, nb * 512 : (nb + 1) * 512],
                        start=True, stop=True, skip_group_check=True,
                    )
                    dst = retrrows_sb[:, nb * 512 : (nb + 1) * 512]
                    if nb % 2 == 0:
                        nc.scalar.mul(dst, mps[:], crecip_sb[:, 0:1])
                    else:
                        nc.vector.tensor_scalar_mul(dst, mps[:], crecip_sb[:, 0:1])

                # retrieved rows -> DRAM (three queues shorten the bounce)
                for h in range(4):
                    eng = (nc.sync, nc.scalar, nc.gpsimd, nc.sync)[h]
                    eng.dma_start(
                        rbt[h],
                        retrrows_sb[32 * h : 32 * h + 32, :].rearrange(
                            "c (d e) -> c d e", e=64
                        ),
                    )

        # ---- phase 2 SBUF pools (reuse space freed by xt) ----
        # retrieved states stored block-diagonally per head pair: head 2p+h2's
        # [64d, 64e] tile sits at (64*h2, c*128 + 64*h2); zeros elsewhere let
        # the LT matmuls run paired with full-128 contraction
        anp2 = ctx.enter_context(tc.tile_pool(name="anp2", bufs=1))
        retrtbd = [anp2.tile([128, C * 128], bf, name=f"retrtbd{p}") for p in range(2)]
        an_sb = [anp2.tile([128, DHC], bf, name=f"an{i}") for i in range(C)]
        # zero the block-diag tiles 3-way across DVE/GPS/ACT in parallel
        # (serial DVE memsets here sat on the critical path to the readback);
        # the ACT zero multiplies an arbitrary initialized tile by 0.0
        for p in range(2):
            nc.vector.memset(retrtbd[p][:, 0:2048], 0.0)
            nc.gpsimd.memset(retrtbd[p][:, 2048:3072], 0.0)
            nc.scalar.mul(
                retrtbd[p][:, 3072:4096].rearrange("q (a e) -> q a e", e=512),
                maskb_sb[:, 0:512].rearrange("q (a e) -> q a e", a=1).broadcast_to(
                    [128, 2, 512]
                ),
                0.0,
            )

        # retrT read-back from the bounce (three queues) into the diag blocks
        for p in range(2):
            for h2 in range(2):
                eng = (nc.sync, nc.gpsimd, nc.scalar, nc.sync)[2 * p + h2]
                eng.dma_start(
                    retrtbd[p][64 * h2 : 64 * h2 + 64, :].rearrange(
                        "d (c e) -> d c e", e=128
                    )[:, :, 64 * h2 : 64 * h2 + 64],
                    rbt[2 * p + h2].rearrange("c d e -> d c e"),
                )

        # ---- pass 1: local scores/exp/mask (A) + U/LT (B) + combine (B2) ----
        # Engine economics (measured): ACT/DVE/GPS ops cost ~300-1100ns each,
        # mostly fixed, so per-chunk op count rules. Plan:
        #   PE:  st x4, U x4 (N=65: rowsums ride the v ones column),
        #        LT x2 (paired, block-diag retrtbd, full-128 contraction)
        #   ACT: exp, u-evac
        #   DVE: recip + bcast-mul + 2 adds (combine)
        #   GPS: mask-mul
        # ult is ONE PSUM bank per chunk: U 4x65 at cols 0:260, LT pair0 at
        # 260:388; LT pair1 rotates through a shared aux bank 4 chunks deep.
        LEAD = 10
        with (
            tc.tile_pool(name="stp", bufs=2, space="PSUM") as stp,
            tc.tile_pool(name="ultp", bufs=3, space="PSUM") as ultp,
            tc.tile_pool(name="auxp", bufs=1, space="PSUM") as auxp,
            tc.tile_pool(name="expr", bufs=3) as exprp,
            tc.tile_pool(name="exps", bufs=LEAD + 3) as expp,
            tc.tile_pool(name="smalls", bufs=4) as smalls,
        ):
            aux = auxp.tile([128, 512], f32, tag="aux")
            exp_q = {}

            def emit_A(i):
                sl = slice(i * 128, (i + 1) * 128)
                st = stp.tile([128, 1024], f32, tag="st")
                for h in HORD:
                    hp, hb = h // 2, 64 * (h % 2)
                    g, b = POS[h]
                    nc.tensor.matmul(
                        st[:, g * 512 + b * 128 : g * 512 + (b + 1) * 128],
                        kt_sb[hp][hb : hb + 64, sl],
                        qt_sb[hp][hb : hb + 64, sl],
                        start=(b == 0), stop=(b == 1), skip_group_check=True,
                    )
                stv = st.rearrange("p (g c) -> p g c", c=512)[:, :, 0:256].rearrange(
                    "p g (b e) -> p g b e", e=128
                )
                expraw = exprp.tile([128, 512], bf, tag="expraw")
                nc.scalar.activation(expraw[:], stv, Exp, scale=0.125)
                expst = expp.tile([128, 512], bf, tag="expst")
                # DVE is idle during the lead-in; GPS handles steady state
                meng = nc.vector if i < LEAD else nc.gpsimd
                meng.tensor_mul(expst[:], expraw[:], maskb_sb[:])
                exp_q[i] = expst

            b_q = {}

            def emit_B(i):
                expst = exp_q.pop(i)
                sl = slice(i * 128, (i + 1) * 128)
                ult = ultp.tile([128, 512], f32, tag="ult")

                # U with fused rowsums: N=65 streams over v|ones, 65-pitch
                for h in range(4):
                    g, b = POS[h]
                    pos = g * 2 + b
                    nc.tensor.matmul(
                        ult[:, pos * 65 : (pos + 1) * 65],
                        expst[:, pos * 128 : (pos + 1) * 128],
                        v_sb[i].rearrange("p (h e) -> p h e", e=65)[:, h, 0:65],
                        start=(h == 0), stop=(h == 3), skip_group_check=True,
                    )
                # paired LT: block-diag retrieved states, full-K, full-row
                rv0 = retrtbd[0][:].rearrange("d (c e) -> d c e", e=128)
                nc.tensor.matmul(
                    ult[:, 260:388], qt_sb[0][:, sl], rv0[:, i, :],
                    start=True, stop=True, skip_group_check=True,
                )
                rv1 = retrtbd[1][:].rearrange("d (c e) -> d c e", e=128)
                aux_c = (i % 4) * 128
                nc.tensor.matmul(
                    aux[:, aux_c : aux_c + 128], qt_sb[1][:, sl], rv1[:, i, :],
                    start=True, stop=True, skip_group_check=True,
                )
                u_sbuf = smalls.tile([128, 260], bf, tag="usb")
                nc.scalar.copy(u_sbuf[:], ult[:, 0:260])
                rr = smalls.tile([128, 4], f32, tag="rr")
                nc.vector.reciprocal(
                    rr[:],
                    ult[:, 0:260].rearrange("p (h e) -> p h e", e=65)[:, :, 64:65],
                )
                b_q[i] = (ult, aux_c, u_sbuf, rr)

            def emit_B2(i):
                # lagged combine: an = u * rr + lt, POS head order throughout
                # (host permutes Wo rows to match). rr broadcasts along the
                # free axis (stride-0 AP), then the two LT pairs add in.
                # pair0 = heads 0,1 = pos blocks 0,2; pair1 = pos blocks 1,3.
                ult, aux_c, u_sbuf, rr = b_q.pop(i)
                anv = an_sb[i][:].rearrange("p (h e) -> p h e", e=64)
                nc.vector.tensor_mul(
                    anv,
                    u_sbuf[:].rearrange("p (h e) -> p h e", e=65)[:, :, 0:64],
                    rr[:].rearrange("p (h e) -> p h e", e=1).broadcast_to(
                        [128, 4, 64]
                    ),
                )
                an2 = an_sb[i][:].rearrange("p (g e) -> p g e", e=128)
                nc.vector.tensor_add(
                    an2[:, :, 0:64], an2[:, :, 0:64],
                    ult[:, 260:388].rearrange("p (g e) -> p g e", e=64),
                )
                nc.vector.tensor_add(
                    an2[:, :, 64:128], an2[:, :, 64:128],
                    aux[:, aux_c : aux_c + 128].rearrange(
                        "p (g e) -> p g e", e=64
                    ),
                )

            for i in range(C):
                emit_A(i)
                if i > LEAD:
                    emit_B2(i - LEAD - 1)
                if i >= LEAD:
                    emit_B(i - LEAD)
            for i in range(C - LEAD, C):
                emit_B2(i - 1)
                emit_B(i)
            emit_B2(C - 1)

        # ---- pass 2: transpose + output projection + DMA out ----
        # out-projection lags the transpose by one chunk so the PE never
        # waits on the attnt evacuation
        with (
            tc.tile_pool(name="tpp", bufs=2, space="PSUM") as tpp,
            tc.tile_pool(name="outp", bufs=3, space="PSUM") as outp,
            tc.tile_pool(name="attp", bufs=3) as attp,
            tc.tile_pool(name="outs", bufs=3) as outsp,
        ):
            att_q = {}
            osb_hold = [None]

            def emit_T(i):
                tp = tpp.tile([128, 256], bf, tag="tp")
                for g in range(2):
                    nc.tensor.transpose(
                        tp[:, g * 128 : (g + 1) * 128],
                        an_sb[i][:, g * 128 : (g + 1) * 128],
                        ident_sb[:],
                    )
                attnt = attp.tile([128, 256], bf, tag="attnt")
                nc.scalar.copy(attnt[:], tp[:])
                att_q[i] = attnt

            def emit_O(i):
                attnt = att_q.pop(i)
                if i % 2 == 0:
                    osb_hold[0] = outsp.tile(
                        [128, 2, D], bf, tag="osb", name="osb2"
                    )
                osb = osb_hold[0]
                ops = outp.tile([128, 1024], f32, tag="ops")
                for nb in range(2):
                    for p in range(2):
                        nc.tensor.matmul(
                            ops[:, nb * 512 : (nb + 1) * 512],
                            attnt[:, p * 128 : (p + 1) * 128],
                            wo_sb[:, p, nb * 512 : (nb + 1) * 512],
                            start=(p == 0),
                            stop=(p == 1),
                        )
                # O evacuation split across DVE and ACT to balance pass 2
                nc.vector.tensor_copy(osb[:, i % 2, 0:512], ops[:, 0:512])
                nc.scalar.copy(osb[:, i % 2, 512:1024], ops[:, 512:1024])
                if i == C - 1:
                    # final chunk: two parallel half-chunk DMAs shorten drain
                    nc.sync.dma_start(
                        out_ap[i * 128 : (i + 1) * 128, 0:512],
                        osb[:, i % 2, 0:512],
                    )
                    nc.scalar.dma_start(
                        out_ap[i * 128 : (i + 1) * 128, 512:1024],
                        osb[:, i % 2, 512:1024],
                    )
                elif i >= C - 4:
                    # penultimate chunks: eager single-chunk DMAs
                    eng = (nc.sync, nc.scalar, nc.gpsimd)[i % 3]
                    eng.dma_start(
                        out_ap[i * 128 : (i + 1) * 128, :], osb[:, i % 2, :]
                    )
                elif i % 2 == 1:
                    eng = (nc.sync, nc.gpsimd, nc.scalar)[(i // 2) % 3]
                    eng.dma_start(
                        out_ap[(i - 1) * 128 : (i + 1) * 128, :].rearrange(
                            "(c p) j -> p c j", p=128
                        ),
                        osb[:],
                    )

            for i in range(C):
                emit_T(i)
                if i >= 1:
                    emit_O(i - 1)
            emit_O(C - 1)

    _split_heavy_waits(nc)
    return nc


_CACHE = {}


def _get_program():
    if "nc" not in _CACHE:
        _CACHE["nc"] = _build_program()
    return _CACHE["nc"]


def _make_in_maps(inputs):
    hs = np.asarray(inputs["hidden_states"], dtype=np.float32)
    W = {k: np.asarray(inputs[k], dtype=np.float32) for k in
         ("Wq", "Wk", "Wv", "Wo", "Wr", "Wh")}
    bvec = {k: np.asarray(inputs[k], dtype=np.float32) for k in
            ("bq", "bk", "bv", "bo", "br", "bh")}

    # local binary mask in [key, query] layout (keep key >= query), tiled x4
    tt, ss = np.meshgrid(np.arange(128), np.arange(128), indexing="ij")
    maskb = (tt >= ss).astype(np.float32)
    maskb4 = np.tile(maskb, (1, 4)).astype(BF16)
    # chunk mask in [j, c'] layout: keep j >= c'; tiled x4 heads (additive)
    cc2, cc = np.meshgrid(np.arange(C), np.arange(C), indexing="ij")
    cmask = np.where(cc2 >= cc, 0.0, NEG).astype(np.float32)
    cmask4 = np.tile(cmask, (4, 1)).astype(np.float32)
    ident = np.eye(128, dtype=np.float32).astype(BF16)

    xT_bf = [np.ascontiguousarray(hs[b].T).astype(BF16) for b in range(B)]
    # attn rows reach the out projection in POS head order (h0,h2,h1,h3)
    POS_PERM = np.concatenate(
        [np.arange(64 * h, 64 * h + 64) for h in (0, 2, 1, 3)]
    )

    in_maps = []
    for c in range(NCORES):
        b, hg = divmod(c, 4)
        sl = slice(hg * DHC, (hg + 1) * DHC)
        bias = np.stack(
            [
                bvec["bq"][sl][:128], bvec["bq"][sl][128:],
                bvec["bk"][sl][:128], bvec["bk"][sl][128:],
                bvec["br"][sl][:128], bvec["br"][sl][128:],
                bvec["bh"][sl][:128], bvec["bh"][sl][128:],
            ],
            axis=1,
        ).astype(np.float32)
        bvkrow = np.concatenate([bvec["bv"][sl], bvec["bk"][sl]])[None, :].astype(BF16)
        in_maps.append(
            {
                "xT": xT_bf[b],
                "wqT": np.ascontiguousarray(W["Wq"][sl, :].T).astype(BF16),
                "wkT": np.ascontiguousarray(W["Wk"][sl, :].T).astype(BF16),
                "wvT": np.ascontiguousarray(W["Wv"][sl, :].T).astype(BF16),
                "wrT": np.ascontiguousarray(W["Wr"][sl, :].T).astype(BF16),
                "whT": np.ascontiguousarray(W["Wh"][sl, :].T).astype(BF16),
                "woT": np.ascontiguousarray(
                    W["Wo"][:, sl].T[POS_PERM, :]
                ).astype(BF16),
                "bias": bias,
                "bvkrow": bvkrow,
                "ident": ident,
                "maskb4": maskb4,
                "cmask4": cmask4,
            }
        )
    return in_maps, bvec["bo"]


def kernel(**inputs):
    nc = _get_program()
    in_maps, bo = _make_in_maps(inputs)
    res = run_bass_kernel_spmd(nc, in_maps, core_ids=list(range(NCORES)))
    _CACHE["last_results"] = res
    out = np.zeros((B, S, D), np.float32)
    for c in range(NCORES):
        out[c // 4] += np.asarray(res.results[c]["out"], dtype=np.float32)
    out += bo[None, None, :]
    return out

